# revision 4
# baseline (speedup 1.0000x reference)
"""Trainium2 Bass kernel for nn_CurvedMultiHeadAttention (B=4, S=1024, E=768, H=12, D=64, R=16).

Sharding: 8 cores; core c handles batch b=c//2 and heads h0=6*(c%2) .. h0+5.
Each core computes its 6 heads' out-projection contribution; the host sums the
two partials per batch element and adds bo (the unshard step).

Math (validated in the previous baseline at ~2e-3 rel err):
 - softmax over keys is invariant to per-query shifts => qq term drops.
 - EPS*I part of G_h contributes <1e-5 to scores => dropped.
 - scoresT[t,s] = sum_r kA[r,t]*qA[r,s]; per-key bias ckk[t] = -SCALE*kk[t]
   + mask[t] rides the ScalarE exp as the partition-axis bias.
 - Weff = [A^T Wq ; A^T bq] is folded on the HOST (weight preprocessing),
   so the q/k projection emits 16 rows per head directly.

Key structure (this version):
 - qk-projection emits 4 m-tiles laid out so head h's qA and kA both sit at
   partition offset 32*g(h); the K=16 scores (and kk) matmuls then run
   4-way ROW-TILED via tile_position=(32g,0) - ~3x PE concurrency.
 - ctx is computed TRANSPOSED: stationary [v_h | 1] (M=65), streaming the
   exp weights. Row 64 of the PSUM result is the softmax denominator; the
   evacuation is a single fused DVE divide against a GpSimd partition-
   broadcast of that row. No PE transposes, no per-(h,s) reciprocals.
 - out-projection consumes ctxT tiles directly as the stationary operand.
All heavy matmuls bf16 with fp32 PSUM accumulation.
"""

import os
import numpy as np
import ml_dtypes

import concourse.bass as bass
import concourse.tile as tile
from concourse import bacc
from concourse import mybir
from concourse.bass_utils import run_bass_kernel_spmd

F32 = mybir.dt.float32
BF16 = mybir.dt.bfloat16
AF = mybir.ActivationFunctionType
ALU = mybir.AluOpType

S = 1024          # sequence length
E = 768           # embed
D = 64            # head dim
R = 16            # rank
HPC = 6           # heads per core
NCORES = 8
SCALE = 1.0 / 8.0
ESC = 2.0 * SCALE  # exp scale
EAUG = E + 1       # ones row folds biases into the projections
KCH = [128] * 6 + [1]
NT = S // 128      # 8 key/query chunks

GRP = [0, 1, 2, 3, 0, 1]      # PE row-group per head
QT = [0, 0, 0, 0, 2, 2]       # qkb tile index holding qA of head h (kA: +1)
POF = [0, 32, 64, 96, 0, 32]  # partition offset of the head's 16 rows

LAST_RESULTS = None


def _emit(tc):
    nc = tc.nc
    hTd = nc.dram_tensor("hTa", [EAUG, S], BF16, kind="ExternalInput")
    wqd = nc.dram_tensor("WeffT", [EAUG, 512], BF16, kind="ExternalInput")
    wvd = nc.dram_tensor("WvTa", [EAUG, HPC * D], BF16, kind="ExternalInput")
    wod = nc.dram_tensor("WoT", [HPC * D, E], BF16, kind="ExternalInput")
    mkd = nc.dram_tensor("mask48", [128, 48], F32, kind="ExternalInput")
    outd = nc.dram_tensor("outp", [S, E], F32, kind="ExternalOutput")

    import contextlib
    stack = contextlib.ExitStack()
    const = stack.enter_context(tc.tile_pool(name="const", bufs=1))
    work = stack.enter_context(tc.tile_pool(name="work", bufs=2))
    ptp = stack.enter_context(tc.tile_pool(name="ptp", bufs=2))
    psb = stack.enter_context(tc.tile_pool(name="psb", bufs=2, space="PSUM"))
    pct = stack.enter_context(tc.tile_pool(name="pct", bufs=1, space="PSUM"))
    psm = stack.enter_context(tc.tile_pool(name="psm", bufs=2, space="PSUM"))

    dma = nc.sync.dma_start
    cp = nc.vector.tensor_copy
    mm = nc.tensor.matmul

    # ---------------- constant / weight loads ----------------
    hT, wqk, wv = [], [], []
    for i, kc in enumerate(KCH):
        r0 = 128 * i
        hT.append(const.tile([kc, S], BF16, name=f"hT{i}", tag=f"hT{i}"))
        dma(out=hT[i][:, :], in_=hTd[r0:r0 + kc, :])
        wqk.append(const.tile([kc, 512], BF16, name=f"wq{i}", tag=f"wq{i}"))
        dma(out=wqk[i][:, :], in_=wqd[r0:r0 + kc, :])
    mask48 = const.tile([128, 48], F32, name="mask48", tag="mask48")
    dma(out=mask48[:, :], in_=mkd[:, :])
    for i, kc in enumerate(KCH):
        r0 = 128 * i
        wv.append(const.tile([kc, HPC * D], BF16, name=f"wv{i}", tag=f"wv{i}"))
        dma(out=wv[i][:, :], in_=wvd[r0:r0 + kc, :])
    wo = []
    for j in range(3):
        wo.append(const.tile([128, E], BF16, name=f"wo{j}", tag=f"wo{j}"))
        dma(out=wo[j][:, :], in_=wod[128 * j:128 * (j + 1), :])

    ones128 = const.tile([128, 1], BF16, name="ones128", tag="ones128")
    nc.vector.memset(ones128[:, :], 1.0)

    qkb = [const.tile([128, S], BF16, name=f"qkb{m}", tag=f"qkb{m}")
           for m in range(4)]
    ksqA = const.tile([128, S], BF16, name="ksqA", tag="ksqA")
    ksqB = const.tile([64, S], BF16, name="ksqB", tag="ksqB")
    ckkT = const.tile([128, 48], F32, name="ckkT", tag="ckkT")
    vsb = [const.tile([128, HPC * (D + 1)], BF16, name=f"vsb{j}", tag=f"vsb{j}")
           for j in range(NT)]
    ctxT = [const.tile([128, S], BF16, name=f"ctxT{j}", tag=f"ctxT{j}")
            for j in range(3)]

    # ---------------- qk projection: 4 m-tiles of Weff.T @ hTa -------------
    for m in range(4):
        pq = psb.tile([128, S], F32, name="psb", tag="psb")
        for k in range(7):
            for n in range(2):
                mm(out=pq[:, 512 * n:512 * (n + 1)],
                   lhsT=wqk[k][:, 128 * m:128 * (m + 1)],
                   rhs=hT[k][:, 512 * n:512 * (n + 1)],
                   start=(k == 0), stop=(k == 6))
        cp(qkb[m][:, :], pq[:, :])

    # ---------------- kk per key + exp bias ckkT ----------------
    for h in range(HPC):
        p0 = POF[h]
        kt = qkb[QT[h] + 1]
        dst = ksqA if h < 4 else ksqB
        nc.vector.tensor_mul(dst[p0:p0 + 16, :], kt[p0:p0 + 16, :],
                             kt[p0:p0 + 16, :])
    pkk = psm.tile([128, 512], F32, name="psm", tag="psm")
    for h in range(HPC):
        g, p0 = GRP[h], POF[h]
        src = ksqA if h < 4 else ksqB
        for j in range(NT):
            mm(out=pkk[:, 8 * h + j:8 * h + j + 1],
               lhsT=src[p0:p0 + 16, 128 * j:128 * (j + 1)],
               rhs=ones128[32 * g:32 * g + 16, :], start=True, stop=True,
               tile_position=(32 * g, 0))
    nc.vector.scalar_tensor_tensor(
        out=ckkT[:, :], in0=pkk[:, 0:48], scalar=-SCALE, in1=mask48[:, :],
        op0=ALU.mult, op1=ALU.add)

    # ---------------- v projection -> vsb (ones col interleaved) ----------
    for j in range(NT):
        pv = psm.tile([128, 512], F32, name="psm", tag="psm")
        for k in range(7):
            mm(out=pv[:, 0:HPC * D], lhsT=hT[k][:, 128 * j:128 * (j + 1)],
               rhs=wv[k][:, :], start=(k == 0), stop=(k == 6))
        vv = vsb[j][:, :].rearrange("p (h c) -> p h c", h=HPC)
        cp(vv[:, :, 0:D], pv[:, 0:HPC * D].rearrange("p (h d) -> p h d", h=HPC))
        nc.vector.memset(vv[:, :, D:D + 1], 1.0)

    # ---------------- per-head attention ----------------
    for h in range(HPC):
        g, p0 = GRP[h], POF[h]
        qt, kt = qkb[QT[h]], qkb[QT[h] + 1]
        ptst = ptp.tile([128, NT * S], BF16, name="pts", tag="pts")
        ptsv = ptst[:, :].rearrange("p (j n) -> p j n", j=NT)
        for j in range(NT):
            psc = psb.tile([128, S], F32, name="psb", tag="psb")
            for n in range(2):
                mm(out=psc[:, 512 * n:512 * (n + 1)],
                   lhsT=kt[p0:p0 + 16, 128 * j:128 * (j + 1)],
                   rhs=qt[p0:p0 + 16, 512 * n:512 * (n + 1)],
                   start=True, stop=True, tile_position=(32 * g, 0))
            nc.scalar.activation(out=ptsv[:, j, :], in_=psc[:, :], func=AF.Exp,
                                 bias=ckkT[:, 8 * h + j:8 * h + j + 1],
                                 scale=ESC)
        pc = pct.tile([128, S], F32, name="pct", tag="pct")
        for j in range(NT):
            for n in range(2):
                mm(out=pc[0:D + 1, 512 * n:512 * (n + 1)],
                   lhsT=vsb[j][:, (D + 1) * h:(D + 1) * (h + 1)],
                   rhs=ptsv[:, j, 512 * n:512 * (n + 1)],
                   start=(j == 0), stop=(j == NT - 1))
        rr = work.tile([1, S], F32, name="rr", tag="rr")
        nc.vector.reciprocal(rr[:, :], pc[D:D + 1, :])
        rbc = work.tile([64, S], F32, name="rbc", tag="rbc")
        nc.gpsimd.partition_broadcast(rbc[:, :], rr[:, :])
        nc.vector.tensor_mul(
            ctxT[h // 2][64 * (h % 2):64 * (h % 2) + 64, :],
            pc[0:D, :], rbc[:, :])

    # ---------------- out projection + store -------------
    for s in range(NT):
        po = psb.tile([128, S], F32, name="psb", tag="psb")
        for kc in range(3):
            for n0, nw in ((0, 512), (512, 256)):
                mm(out=po[:, n0:n0 + nw],
                   lhsT=ctxT[kc][:, 128 * s:128 * (s + 1)],
                   rhs=wo[kc][:, n0:n0 + nw], start=(kc == 0), stop=(kc == 2))
        osb = work.tile([128, E], F32, name="osb", tag="osb")
        cp(osb[:, :], po[:, 0:E])
        dma(out=outd[128 * s:128 * (s + 1), :], in_=osb[:, :])

    stack.close()


_NC_CACHE = None


def _build():
    global _NC_CACHE
    if _NC_CACHE is None:
        nc = bacc.Bacc("TRN2", target_bir_lowering=False, debug=False,
                       enable_asserts=True, num_devices=NCORES)
        with tile.TileContext(nc) as tc:
            _emit(tc)
        nc.compile()
        _NC_CACHE = nc
    return _NC_CACHE


def kernel(hidden_states, attention_mask, Wq, bq, Wk, bk, Wv, bv, Wo, bo, A,
           **_ignored):
    global LAST_RESULTS
    hidden_states = np.asarray(hidden_states, np.float32)
    attention_mask = np.asarray(attention_mask, np.float32)
    Wq, bq = np.asarray(Wq, np.float32), np.asarray(bq, np.float32)
    Wk, bk = np.asarray(Wk, np.float32), np.asarray(bk, np.float32)
    Wv, bv = np.asarray(Wv, np.float32), np.asarray(bv, np.float32)
    Wo, bo = np.asarray(Wo, np.float32), np.asarray(bo, np.float32)
    A = np.asarray(A, np.float32)

    B = hidden_states.shape[0]
    nc = _build()

    bf = ml_dtypes.bfloat16
    ones1 = np.ones((1, S), np.float32)
    in_maps = []
    for c in range(NCORES):
        b = c // 2
        h0 = HPC * (c % 2)
        sl = slice(h0 * D, (h0 + HPC) * D)
        hTa = np.concatenate([hidden_states[b].T, ones1], 0)

        Weff = np.zeros((EAUG, 512), np.float32)
        for h in range(HPC):
            gh = h0 + h
            Ah = A[gh]                                   # (64, 16)
            Weq = Ah.T @ Wq[D * gh:D * (gh + 1), :]      # (16, 768)
            beq = Ah.T @ bq[D * gh:D * (gh + 1)]
            Wek = Ah.T @ Wk[D * gh:D * (gh + 1), :]
            bek = Ah.T @ bk[D * gh:D * (gh + 1)]
            if h < 4:
                qc, kc2 = 32 * h, 128 + 32 * h
            else:
                qc, kc2 = 256 + 32 * (h - 4), 384 + 32 * (h - 4)
            Weff[0:E, qc:qc + R] = Weq.T
            Weff[E, qc:qc + R] = beq
            Weff[0:E, kc2:kc2 + R] = Wek.T
            Weff[E, kc2:kc2 + R] = bek

        WvTa = np.concatenate([Wv[sl].T, bv[sl][None, :]], 0)     # (769, 384)
        WoT = Wo[:, sl].T.copy()                                  # (384, 768)
        maskT = attention_mask[b, 0, 0].reshape(NT, 128).T        # (128, 8)
        mask48 = np.tile(maskT, (1, HPC))                         # (128, 48)
        in_maps.append({
            "hTa": np.ascontiguousarray(hTa.astype(bf)),
            "WeffT": np.ascontiguousarray(Weff.astype(bf)),
            "WvTa": np.ascontiguousarray(WvTa.astype(bf)),
            "WoT": np.ascontiguousarray(WoT.astype(bf)),
            "mask48": np.ascontiguousarray(mask48),
        })

    res = run_bass_kernel_spmd(nc, in_maps, list(range(NCORES)),
                               trace=bool(os.environ.get("KERNEL_TRACE")))
    LAST_RESULTS = res
    parts = [res.results[c]["outp"] for c in range(NCORES)]
    out = np.stack([parts[2 * b] + parts[2 * b + 1] + bo[None, :]
                    for b in range(B)], 0)
    return np.ascontiguousarray(out.astype(np.float32))


# revision 18
# speedup vs baseline: 1.1898x; 1.1898x over previous
"""Trainium2 Bass kernel for nn_CurvedMultiHeadAttention (B=4, S=1024, E=768, H=12, D=64, R=16).

Sharding: 8 cores; core c handles batch b=c//2 and heads h0=6*(c%2) .. h0+5.
Each core computes its 6 heads' out-projection contribution; the host sums the
two partials per batch element and adds bo (the unshard step).

Math (validated in the previous baseline at ~2e-3 rel err):
 - softmax over keys is invariant to per-query shifts => qq term drops.
 - EPS*I part of G_h contributes <1e-5 to scores => dropped.
 - scoresT[t,s] = sum_r kA[r,t]*qA[r,s]; per-key bias ckk[t] = -SCALE*kk[t]
   + mask[t] rides the ScalarE exp as the partition-axis bias.
 - Weff = [A^T Wq ; A^T bq] is folded on the HOST (weight preprocessing),
   so the q/k projection emits 16 rows per head directly.

Key structure (this version):
 - qk-projection emits 4 m-tiles laid out so head h's qA and kA both sit at
   partition offset 32*g(h); the K=16 scores (and kk) matmuls then run
   4-way ROW-TILED via tile_position=(32g,0) - ~3x PE concurrency.
 - ctx is computed TRANSPOSED: stationary [v_h | 1] (M=65), streaming the
   exp weights. Row 64 of the PSUM result is the softmax denominator; the
   evacuation is a single fused DVE divide against a GpSimd partition-
   broadcast of that row. No PE transposes, no per-(h,s) reciprocals.
 - out-projection consumes ctxT tiles directly as the stationary operand.
All heavy matmuls bf16 with fp32 PSUM accumulation.
"""

import os
import numpy as np
import ml_dtypes

import concourse.bass as bass
import concourse.tile as tile
from concourse import bacc
from concourse import mybir
from concourse.bass_utils import run_bass_kernel_spmd

F32 = mybir.dt.float32
BF16 = mybir.dt.bfloat16
AF = mybir.ActivationFunctionType
ALU = mybir.AluOpType

S = 1024          # sequence length
E = 768           # embed
D = 64            # head dim
R = 16            # rank
HPC = 6           # heads per core
NCORES = 8
SCALE = 1.0 / 8.0
ESC = 2.0 * SCALE  # exp scale
EAUG = E + 1       # ones row folds biases into the projections
KCH = [128] * 6 + [1]
NT = S // 128      # 8 key/query chunks

GRP = [0, 1, 2, 3, 0, 1]      # PE row-group per head
QT = [0, 0, 0, 0, 2, 2]       # qkb tile index holding qA of head h (kA: +1)
POF = [0, 32, 64, 96, 0, 32]  # partition offset of the head's 16 rows

LAST_RESULTS = None


def _emit(tc):
    nc = tc.nc
    hTd = nc.dram_tensor("hTa", [EAUG, S], BF16, kind="ExternalInput")
    wqd = nc.dram_tensor("WeffT", [EAUG, 512], BF16, kind="ExternalInput")
    wvd = nc.dram_tensor("WvTa", [EAUG, HPC * D], BF16, kind="ExternalInput")
    wod = nc.dram_tensor("WoT", [HPC * D, E], BF16, kind="ExternalInput")
    mkd = nc.dram_tensor("mask48", [128, 48], F32, kind="ExternalInput")
    outd = nc.dram_tensor("outp", [S, E], F32, kind="ExternalOutput")

    import contextlib
    stack = contextlib.ExitStack()
    const = stack.enter_context(tc.tile_pool(name="const", bufs=1))
    work = stack.enter_context(tc.tile_pool(name="work", bufs=2))
    ptp = stack.enter_context(tc.tile_pool(name="ptp", bufs=3))
    psb = stack.enter_context(tc.tile_pool(name="psb", bufs=2, space="PSUM"))
    pct = stack.enter_context(tc.tile_pool(name="pct", bufs=1, space="PSUM"))
    psm = stack.enter_context(tc.tile_pool(name="psm", bufs=2, space="PSUM"))

    dma = nc.sync.dma_start
    cp = nc.vector.tensor_copy
    mm = nc.tensor.matmul

    # ---------------- constant / weight loads ----------------
    hT, wqk, wv = [], [], []
    for i, kc in enumerate(KCH):
        r0 = 128 * i
        hT.append(const.tile([kc, S], BF16, name=f"hT{i}", tag=f"hT{i}"))
        dma(out=hT[i][:, :], in_=hTd[r0:r0 + kc, :])
        wqk.append(const.tile([kc, 512], BF16, name=f"wq{i}", tag=f"wq{i}"))
        dma(out=wqk[i][:, :], in_=wqd[r0:r0 + kc, :])
    mask48 = const.tile([128, 48], F32, name="mask48", tag="mask48")
    dma(out=mask48[:, :], in_=mkd[:, :])
    for i, kc in enumerate(KCH):
        r0 = 128 * i
        wv.append(const.tile([kc, HPC * D], BF16, name=f"wv{i}", tag=f"wv{i}"))
        dma(out=wv[i][:, :], in_=wvd[r0:r0 + kc, :])
    wo = []
    for j in range(3):
        wo.append(const.tile([128, E], BF16, name=f"wo{j}", tag=f"wo{j}"))
        dma(out=wo[j][:, :], in_=wod[128 * j:128 * (j + 1), :])

    ones128 = const.tile([128, 1], BF16, name="ones128", tag="ones128")
    nc.vector.memset(ones128[:, :], 1.0)

    qkb = [const.tile([128, S], BF16, name=f"qkb{m}", tag=f"qkb{m}")
           for m in range(4)]
    ksqA = const.tile([128, S], BF16, name="ksqA", tag="ksqA")
    ksqB = const.tile([64, S], BF16, name="ksqB", tag="ksqB")
    ckkT = const.tile([128, 48], F32, name="ckkT", tag="ckkT")
    vsb = [const.tile([128, HPC * (D + 1)], BF16, name=f"vsb{j}", tag=f"vsb{j}")
           for j in range(NT)]
    ctxT = [const.tile([128, S], BF16, name=f"ctxT{j}", tag=f"ctxT{j}")
            for j in range(3)]

    # ---------------- qk projection: 4 m-tiles of Weff.T @ hTa -------------
    for m in range(4):
        pq = psb.tile([128, S], F32, name="psb", tag="psb")
        for k in range(7):
            for n in range(2):
                mm(out=pq[:, 512 * n:512 * (n + 1)],
                   lhsT=wqk[k][:, 128 * m:128 * (m + 1)],
                   rhs=hT[k][:, 512 * n:512 * (n + 1)],
                   start=(k == 0), stop=(k == 6))
        cp(qkb[m][:, :], pq[:, :])

    # ---------------- kk per key + exp bias ckkT ----------------
    for h in range(HPC):
        p0 = POF[h]
        kt = qkb[QT[h] + 1]
        dst = ksqA if h < 4 else ksqB
        nc.vector.tensor_mul(dst[p0:p0 + 16, :], kt[p0:p0 + 16, :],
                             kt[p0:p0 + 16, :])
    pkk = psm.tile([128, 512], F32, name="psm", tag="psm")
    for h in range(HPC):
        g, p0 = GRP[h], POF[h]
        src = ksqA if h < 4 else ksqB
        for j in range(NT):
            mm(out=pkk[:, 8 * h + j:8 * h + j + 1],
               lhsT=src[p0:p0 + 16, 128 * j:128 * (j + 1)],
               rhs=ones128[32 * g:32 * g + 16, :], start=True, stop=True,
               tile_position=(32 * g, 0))
    nc.vector.scalar_tensor_tensor(
        out=ckkT[:, :], in0=pkk[:, 0:48], scalar=-SCALE, in1=mask48[:, :],
        op0=ALU.mult, op1=ALU.add)

    # ---------------- v projection -> vsb (ones col interleaved) ----------
    for j in range(NT):
        pv = psm.tile([128, 512], F32, name="psm", tag="psm")
        for k in range(7):
            mm(out=pv[:, 0:HPC * D], lhsT=hT[k][:, 128 * j:128 * (j + 1)],
               rhs=wv[k][:, :], start=(k == 0), stop=(k == 6))
        vv = vsb[j][:, :].rearrange("p (h c) -> p h c", h=HPC)
        cp(vv[:, :, 0:D], pv[:, 0:HPC * D].rearrange("p (h d) -> p h d", h=HPC))
        nc.vector.memset(vv[:, :, D:D + 1], 1.0)

    # ---------------- per-head attention, head-pair interleaved ----------
    # Heads of a pair sit at different PE row groups, so interleaving their
    # scores matmuls makes the K=16 matmuls run concurrently on the PE.
    den_a = const.tile([4, S], F32, name="den_a", tag="den_a")
    den_b = const.tile([2, S], F32, name="den_b", tag="den_b")
    rec_a = const.tile([4, S], F32, name="rec_a", tag="rec_a")
    rec_b = const.tile([2, S], F32, name="rec_b", tag="rec_b")
    recs = [const.tile([1, S], F32, name=f"rec_{h}", tag=f"rec_{h}")
            for h in range(HPC)]
    cus = [const.tile([D + 1, S], F32, name=f"cu{h}", tag=f"cu{h}")
           for h in range(HPC)]

    def ctx_head(h, ptsv):
        pc = pct.tile([128, S], F32, name="pct", tag="pct")
        for j in range(NT):
            for n in range(2):
                mm(out=pc[0:D + 1, 512 * n:512 * (n + 1)],
                   lhsT=vsb[j][:, (D + 1) * h:(D + 1) * (h + 1)],
                   rhs=ptsv[:, j, 512 * n:512 * (n + 1)],
                   start=(j == 0), stop=(j == NT - 1))
        cp(cus[h][:, :], pc[0:D + 1, :])
        if h < 4:
            dma(out=den_a[h:h + 1, :], in_=cus[h][D:D + 1, :])
        else:
            dma(out=den_b[h - 4:h - 3, :], in_=cus[h][D:D + 1, :])

    def norm_head(h):
        if h < 4:
            dma(out=recs[h][:, :], in_=rec_a[h:h + 1, :])
        else:
            dma(out=recs[h][:, :], in_=rec_b[h - 4:h - 3, :])
        rbc = work.tile([64, S], F32, name="rbc", tag="rbc")
        nc.gpsimd.partition_broadcast(rbc[:, :], recs[h][:, :])
        nc.vector.tensor_mul(
            ctxT[h // 2][64 * (h % 2):64 * (h % 2) + 64, :],
            cus[h][0:D, :], rbc[:, :])

    for hp in range(HPC // 2):
        ha, hb = 2 * hp, 2 * hp + 1
        ptsa = ptp.tile([128, NT * S], BF16, name="pts", tag="pts")
        ptsb = ptp.tile([128, NT * S], BF16, name="pts", tag="pts")
        pva = ptsa[:, :].rearrange("p (j n) -> p j n", j=NT)
        pvb = ptsb[:, :].rearrange("p (j n) -> p j n", j=NT)
        for j in range(NT):
            psca = psb.tile([128, S], F32, name="psb", tag="psb")
            pscb = psb.tile([128, S], F32, name="psb", tag="psb")
            for n in range(2):
                for h, psc in ((ha, psca), (hb, pscb)):
                    g, p0 = GRP[h], POF[h]
                    qt, kt = qkb[QT[h]], qkb[QT[h] + 1]
                    mm(out=psc[:, 512 * n:512 * (n + 1)],
                       lhsT=kt[p0:p0 + 16, 128 * j:128 * (j + 1)],
                       rhs=qt[p0:p0 + 16, 512 * n:512 * (n + 1)],
                       start=True, stop=True, tile_position=(32 * g, 0))
            nc.scalar.activation(out=pva[:, j, :], in_=psca[:, :], func=AF.Exp,
                                 bias=ckkT[:, 8 * ha + j:8 * ha + j + 1],
                                 scale=ESC)
            nc.scalar.activation(out=pvb[:, j, :], in_=pscb[:, :], func=AF.Exp,
                                 bias=ckkT[:, 8 * hb + j:8 * hb + j + 1],
                                 scale=ESC)
        ctx_head(ha, pva)
        ctx_head(hb, pvb)
        if hp == 1:
            nc.vector.reciprocal(rec_a[:, :], den_a[:, :])
            for h in range(4):
                norm_head(h)
        elif hp == 2:
            nc.vector.reciprocal(rec_b[:, :], den_b[:, :])
            norm_head(4)
            norm_head(5)

    # ---------------- out projection + store -------------
    for s in range(NT):
        po = psb.tile([128, S], F32, name="psb", tag="psb")
        for kc in range(3):
            for n0, nw in ((0, 512), (512, 256)):
                mm(out=po[:, n0:n0 + nw],
                   lhsT=ctxT[kc][:, 128 * s:128 * (s + 1)],
                   rhs=wo[kc][:, n0:n0 + nw], start=(kc == 0), stop=(kc == 2))
        osb = work.tile([128, E], F32, name="osb", tag="osb")
        cp(osb[:, :], po[:, 0:E])
        dma(out=outd[128 * s:128 * (s + 1), :], in_=osb[:, :])

    stack.close()


_NC_CACHE = None


def _build():
    global _NC_CACHE
    if _NC_CACHE is None:
        nc = bacc.Bacc("TRN2", target_bir_lowering=False, debug=False,
                       enable_asserts=True, num_devices=NCORES)
        with tile.TileContext(nc) as tc:
            _emit(tc)
        nc.compile()
        _NC_CACHE = nc
    return _NC_CACHE


def kernel(hidden_states, attention_mask, Wq, bq, Wk, bk, Wv, bv, Wo, bo, A,
           **_ignored):
    global LAST_RESULTS
    hidden_states = np.asarray(hidden_states, np.float32)
    attention_mask = np.asarray(attention_mask, np.float32)
    Wq, bq = np.asarray(Wq, np.float32), np.asarray(bq, np.float32)
    Wk, bk = np.asarray(Wk, np.float32), np.asarray(bk, np.float32)
    Wv, bv = np.asarray(Wv, np.float32), np.asarray(bv, np.float32)
    Wo, bo = np.asarray(Wo, np.float32), np.asarray(bo, np.float32)
    A = np.asarray(A, np.float32)

    B = hidden_states.shape[0]
    nc = _build()

    bf = ml_dtypes.bfloat16
    ones1 = np.ones((1, S), np.float32)
    in_maps = []
    for c in range(NCORES):
        b = c // 2
        h0 = HPC * (c % 2)
        sl = slice(h0 * D, (h0 + HPC) * D)
        hTa = np.concatenate([hidden_states[b].T, ones1], 0)

        Weff = np.zeros((EAUG, 512), np.float32)
        for h in range(HPC):
            gh = h0 + h
            Ah = A[gh]                                   # (64, 16)
            Weq = Ah.T @ Wq[D * gh:D * (gh + 1), :]      # (16, 768)
            beq = Ah.T @ bq[D * gh:D * (gh + 1)]
            Wek = Ah.T @ Wk[D * gh:D * (gh + 1), :]
            bek = Ah.T @ bk[D * gh:D * (gh + 1)]
            if h < 4:
                qc, kc2 = 32 * h, 128 + 32 * h
            else:
                qc, kc2 = 256 + 32 * (h - 4), 384 + 32 * (h - 4)
            Weff[0:E, qc:qc + R] = Weq.T
            Weff[E, qc:qc + R] = beq
            Weff[0:E, kc2:kc2 + R] = Wek.T
            Weff[E, kc2:kc2 + R] = bek

        WvTa = np.concatenate([Wv[sl].T, bv[sl][None, :]], 0)     # (769, 384)
        WoT = Wo[:, sl].T.copy()                                  # (384, 768)
        maskT = attention_mask[b, 0, 0].reshape(NT, 128).T        # (128, 8)
        mask48 = np.tile(maskT, (1, HPC))                         # (128, 48)
        in_maps.append({
            "hTa": np.ascontiguousarray(hTa.astype(bf)),
            "WeffT": np.ascontiguousarray(Weff.astype(bf)),
            "WvTa": np.ascontiguousarray(WvTa.astype(bf)),
            "WoT": np.ascontiguousarray(WoT.astype(bf)),
            "mask48": np.ascontiguousarray(mask48),
        })

    res = run_bass_kernel_spmd(nc, in_maps, list(range(NCORES)),
                               trace=bool(os.environ.get("KERNEL_TRACE")))
    LAST_RESULTS = res
    parts = [res.results[c]["outp"] for c in range(NCORES)]
    out = np.stack([parts[2 * b] + parts[2 * b + 1] + bo[None, :]
                    for b in range(B)], 0)
    return np.ascontiguousarray(out.astype(np.float32))
